# revision 25
# baseline (speedup 1.0000x reference)
"""Trainium2 Bass kernel: packed-varlen causal GQA attention block.

Sharding: tensor-parallel across heads on 8 NeuronCores.
  core c: q-heads [4c, 4c+4), kv-head c.
  Phase 1: QKV projection (bf16 matmuls, fp32 accum) + RoPE -> qT/kT [d, tok],
           v [tok, d]. x is staged host-side in 256-token-chunk-major layout so
           each chunk loads with one 16KB-line DMA.
  Phase 2: flash-style attention in transposed layout: ST = K-tile^T stationary
           vs Q moving -> exp -> pT; attT = V-contract(pT); denominator via a
           single ones-matmul per q-block over a DVE-accumulated sum of pT.
  Phase 3: AllGather of attT (bf16), out[:, c*512:(c+1)*512] = att @ wo_cols.
  Schedule: seq-0 attention runs mid-phase-1 (right after its tokens' QKV) so
           its AllGather hides under the remaining projection chunks; each wo
           is deferred one stage so no PE instruction ever waits on a
           collective.
Host only slices/casts/permutes inputs and concatenates the 8 output column
slices.
"""

import sys

import numpy as np
import ml_dtypes

if "/opt/trn_rl_repo" not in sys.path:
    sys.path.insert(0, "/opt/trn_rl_repo")

BF16 = ml_dtypes.bfloat16

# Static problem config (matches the reference).
LENS = [1024, 896, 768, 512]
T = 3200
B = 4
DIM, NH, NKV, HD = 4096, 32, 8, 128
THETA = 500000.0
SCALE = 1.0 / float(np.sqrt(HD))
NCORES = 8
QH = NH // NCORES          # 4 q heads per core
QW = QH * HD               # 512 q/att feature cols per core
KC = DIM // 128            # 32 contraction chunks
SEQ_STARTS = [0, 1024, 1920, 2688]
CHUNK = 256
NCH = (T + CHUNK - 1) // CHUNK        # 13 token chunks (last half-filled)
TP = NCH * CHUNK                      # 3584 padded tokens

_CACHE = {}


def _build_program(phases=(1, 2, 3), collective=True, repeat=1, variant=()):
    import concourse.mybir as mybir
    import concourse.tile as tile
    from concourse import bacc

    f32 = mybir.dt.float32
    bf16 = mybir.dt.bfloat16

    nc = bacc.Bacc("TRN2", target_bir_lowering=False, debug=False,
                   enable_asserts=False, num_devices=NCORES)

    # ---- I/O ----
    xT_d = nc.dram_tensor("xT", [128, NCH, KC, CHUNK], bf16,
                          kind="ExternalInput")
    wq_d = nc.dram_tensor("wq", [128, KC, QW], bf16, kind="ExternalInput")
    wk_d = nc.dram_tensor("wk", [128, KC, HD], bf16, kind="ExternalInput")
    wv_d = nc.dram_tensor("wv", [128, KC, HD], bf16, kind="ExternalInput")
    wo_d = nc.dram_tensor("wo", [128, KC, QW], bf16, kind="ExternalInput")
    cos_d = nc.dram_tensor("cost", [64, TP], f32, kind="ExternalInput")
    sin_d = nc.dram_tensor("sint", [64, TP], f32, kind="ExternalInput")
    tri_d = nc.dram_tensor("tri", [128, 128], bf16, kind="ExternalInput")
    out_d = nc.dram_tensor("out", [T, QW], f32, kind="ExternalOutput")

    with tile.TileContext(nc) as tc:
        with (
            tc.tile_pool(name="sb", bufs=1) as sb,
            tc.tile_pool(name="ps", bufs=2, space="PSUM") as ps,
            tc.tile_pool(name="dram", bufs=1, space="DRAM") as dpool,
        ):
            # ---- resident SBUF tensors ----
            wk_sb = sb.tile([128, KC, HD], bf16)
            wv_sb = sb.tile([128, KC, HD], bf16)
            cos_sb = sb.tile([64, TP], f32)
            sin_sb = sb.tile([64, TP], f32)
            tri_sb = sb.tile([128, 128], bf16)  # 0/1 causal keep-mask
            ones_sb = sb.tile([128, 128], bf16)
            nc.vector.memset(ones_sb[:], 1.0)
            id_sb = sb.tile([128, 128], bf16)
            from concourse.masks import make_identity
            make_identity(nc, id_sb[:])

            qT_sb = sb.tile([128, QH, TP], bf16)  # per q-head [d, tok], roped+scaled
            kT_sb = sb.tile([128, TP], bf16)      # kv head   [d, tok], roped
            v_sb = sb.tile([128, TP], bf16)       # [tok-part, d] per 128-tok tile

            def rope(dst0, dst1, psum, t0, w):
                """dst0/dst1: [64, w] bf16 slices; psum [128, w] f32."""
                p0 = psum[0:64, :]
                p1 = psum[64:128, :]
                cw = cos_sb[:, t0:t0 + w]
                sw = sin_sb[:, t0:t0 + w]
                m0 = sb.tile([64, CHUNK], f32, tag="rtmp", bufs=4)
                nc.vector.tensor_mul(m0[:, :w], p0, cw)
                m1 = sb.tile([64, CHUNK], f32, tag="rtmp", bufs=4)
                nc.vector.tensor_mul(m1[:, :w], p1, sw)
                nc.vector.tensor_sub(dst0, m0[:, :w], m1[:, :w])
                m2 = sb.tile([64, CHUNK], f32, tag="rtmp", bufs=4)
                nc.vector.tensor_mul(m2[:, :w], p0, sw)
                m3 = sb.tile([64, CHUNK], f32, tag="rtmp", bufs=4)
                nc.vector.tensor_mul(m3[:, :w], p1, cw)
                nc.vector.tensor_add(dst1, m2[:, :w], m3[:, :w])

            for _rep in range(repeat):
                wq_sb = sb.tile([128, KC, QW], bf16, tag="bigw")
                nc.sync.dma_start(wq_sb[:, 0:8, :], wq_d.ap()[:, 0:8, :])

                def do_chunk(i, first_rep):
                    t0 = i * CHUNK
                    w = min(CHUNK, T - t0)   # valid tokens in this chunk
                    xt = sb.tile([128, KC, CHUNK], bf16, tag="xt", bufs=2)
                    nc.sync.dma_start(xt[:], xT_d.ap()[:, i])
                    if i == 0:
                        # remaining resident loads, behind the first chunk;
                        # cos/sin before wk so the first rope isn't starved
                        for pc in range(8, KC, 8):
                            nc.sync.dma_start(wq_sb[:, pc:pc + 8, :],
                                              wq_d.ap()[:, pc:pc + 8, :])
                        if first_rep:
                            nc.sync.dma_start(cos_sb[:], cos_d.ap())
                            nc.sync.dma_start(sin_sb[:], sin_d.ap())
                            nc.sync.dma_start(tri_sb[:], tri_d.ap())
                            nc.sync.dma_start(wk_sb[:], wk_d.ap())
                            nc.sync.dma_start(wv_sb[:], wv_d.ap())
                    for h in range(QH):
                        qp = ps.tile([128, 512], f32, tag="A", bufs=3)
                        for kc in range(KC):
                            nc.tensor.matmul(
                                qp[:, :w],
                                wq_sb[:, kc, h * HD:(h + 1) * HD],
                                xt[:, kc, :w],
                                start=(kc == 0), stop=(kc == KC - 1),
                            )
                        rope(qT_sb[0:64, h, t0:t0 + w],
                             qT_sb[64:128, h, t0:t0 + w], qp[:, :w], t0, w)

                    kp = ps.tile([128, 512], f32, tag="A", bufs=3)
                    for kc in range(KC):
                        nc.tensor.matmul(kp[:, :w], wk_sb[:, kc, :],
                                         xt[:, kc, :w],
                                         start=(kc == 0), stop=(kc == KC - 1))
                    rope(kT_sb[0:64, t0:t0 + w],
                         kT_sb[64:128, t0:t0 + w], kp[:, :w], t0, w)

                    # V: vT [d, tok], then PE-transpose each 128-token tile
                    vp = ps.tile([128, 512], f32, tag="A", bufs=3, name="vp")
                    for kc in range(KC):
                        nc.tensor.matmul(vp[:, :w], wv_sb[:, kc, :],
                                         xt[:, kc, :w],
                                         start=(kc == 0), stop=(kc == KC - 1))
                    vt_sb = sb.tile([128, CHUNK], bf16, tag="vt", bufs=1)
                    nc.any.tensor_copy(vt_sb[:, :w], vp[:, :w])
                    for s in range(w // 128):
                        tp = ps.tile([128, 128], bf16, tag="B", bufs=2,
                                     name="tp")
                        nc.tensor.transpose(
                            tp[:], vt_sb[:, s * 128:(s + 1) * 128], id_sb[:])
                        nc.any.tensor_copy(
                            v_sb[:, t0 + s * 128:t0 + (s + 1) * 128], tp[:])

                ag_ins = [dpool.tile([QW, LENS[b]], bf16, tag=f"agin{b}",
                                     name=f"agin{b}")
                          for b in range(B)] if 2 in phases or 3 in phases \
                    else []
                ag_outs = {}

                def do_attn(b):
                    s0 = SEQ_STARTS[b]
                    L = LENS[b]
                    ag_in = ag_ins[b]
                    for h in range(QH):
                        for q0 in range(0, L, 512):
                            w = min(512, L - q0)
                            nkt = (q0 + w) // 128
                            pts = []
                            accs = []
                            for kb in range(nkt):
                                k0 = kb * 128
                                pt = sb.tile([128, 512], bf16, tag="pT",
                                             bufs=12)
                                if k0 + 128 <= q0:
                                    st = ps.tile([128, 512], f32, tag="A",
                                                 bufs=3)
                                    nc.tensor.matmul(
                                        st[:, :w],
                                        kT_sb[:, s0 + k0:s0 + k0 + 128],
                                        qT_sb[:, h, s0 + q0:s0 + q0 + w],
                                        start=True, stop=True)
                                    nc.scalar.activation(
                                        pt[:, :w], st[:, :w],
                                        mybir.ActivationFunctionType.Exp)
                                else:
                                    off = k0 - q0
                                    wd = w - off
                                    st = ps.tile([128, 512], f32, tag="A",
                                                 bufs=3)
                                    nc.tensor.matmul(
                                        st[:, :wd],
                                        kT_sb[:, s0 + k0:s0 + k0 + 128],
                                        qT_sb[:, h, s0 + k0:s0 + k0 + wd],
                                        start=True, stop=True)
                                    wm = min(128, wd)
                                    if off > 0:
                                        nc.vector.memset(pt[:, :off], 0.0)
                                    nc.scalar.activation(
                                        pt[:, off:off + wd], st[:, :wd],
                                        mybir.ActivationFunctionType.Exp)
                                    # zero the invalid triangle post-exp
                                    nc.vector.tensor_mul(
                                        pt[:, off:off + wm],
                                        pt[:, off:off + wm], tri_sb[:, :wm])
                                pts.append(pt)
                                # pairwise DVE pre-sum halves the den matmuls
                                if kb % 2 == 1:
                                    acc = sb.tile([128, 512], bf16, tag="acc",
                                                  bufs=3)
                                    nc.vector.tensor_add(acc[:, :w],
                                                         pts[kb - 1][:, :w],
                                                         pt[:, :w])
                                    accs.append(acc)
                            if nkt % 2 == 1:
                                accs.append(pts[-1])

                            att = ps.tile([128, 512], f32, tag="B", bufs=2)
                            for j in range(nkt):
                                nc.tensor.matmul(
                                    att[:, :w],
                                    v_sb[:, s0 + j * 128:s0 + (j + 1) * 128],
                                    pts[j][:, :w],
                                    start=(j == 0), stop=(j == nkt - 1))
                            den = ps.tile([128, 512], f32, tag="C", bufs=1)
                            for j, a in enumerate(accs):
                                nc.tensor.matmul(den[:, :w], ones_sb[:],
                                                 a[:, :w], start=(j == 0),
                                                 stop=(j == len(accs) - 1))
                            rec = sb.tile([128, 512], f32, tag="rec", bufs=2)
                            nc.vector.reciprocal_approx_fast(rec[:, :w],
                                                             den[:, :w])
                            ao = sb.tile([128, 512], bf16, tag="ao", bufs=2)
                            nc.vector.tensor_mul(ao[:, :w], att[:, :w],
                                                 rec[:, :w])
                            nc.sync.dma_start(
                                ag_in[h * HD:(h + 1) * HD, q0:q0 + w],
                                ao[:, :w])

                    # AllGather for this sequence
                    if 3 in phases:
                        L = LENS[b]
                        ag_out = dpool.tile(
                            [NH * HD, L], bf16, tag=f"agout{b}",
                            name=f"agout{b}",
                            addr_space="Shared" if collective else "Local")
                        if collective:
                            nc.gpsimd.collective_compute(
                                "AllGather",
                                mybir.AluOpType.bypass,
                                replica_groups=[list(range(NCORES))],
                                ins=[ag_in.opt()],
                                outs=[ag_out.opt()],
                            )
                        else:  # single-core sim stand-in: replicate 8x
                            for r in range(NCORES):
                                nc.sync.dma_start(
                                    ag_out[r * QW:(r + 1) * QW, :], ag_in[:])
                        ag_outs[b] = ag_out

                aw_pre = {}

                def prefetch_aw(b, n=2):
                    # issue wo(b)'s first aw loads on the (otherwise idle)
                    # gpsimd queue so the data is resident before the PE
                    # reaches wo(b); the sync queue is clogged with in-order
                    # ao/out DMAs at that point.
                    L = LENS[b]
                    ag_r = ag_outs[b].rearrange("(a p) t -> p a t", p=128)
                    lst = []
                    for t0 in range(0, min(L, n * CHUNK), CHUNK):
                        wl = min(CHUNK, L - t0)
                        aw = sb.tile([128, KC, CHUNK], bf16, tag="aw",
                                     bufs=2, name="aw")
                        nc.gpsimd.dma_start(aw[:, :, :wl],
                                            ag_r[:, :, t0:t0 + wl])
                        lst.append(aw)
                    aw_pre[b] = lst

                def do_wo(b, wo_sb):
                    s0 = SEQ_STARTS[b]
                    L = LENS[b]
                    ag_r = ag_outs[b].rearrange("(a p) t -> p a t", p=128)
                    for ci, t0 in enumerate(range(0, L, CHUNK)):
                        wl = min(CHUNK, L - t0)
                        if ci < len(aw_pre.get(b, ())):
                            aw = aw_pre[b][ci]
                        else:
                            aw = sb.tile([128, KC, CHUNK], bf16, tag="aw",
                                         bufs=2, name="aw")
                            nc.gpsimd.dma_start(aw[:, :, :wl],
                                                ag_r[:, :, t0:t0 + wl])
                        for s in range(wl // 128):
                            op = ps.tile([128, 512], f32, tag="D", bufs=2,
                                         name="op")
                            # two 256-wide moving passes: N=512 matmuls pay a
                            # measured ~90-cycle overhead vs 58 at N=256
                            for half in range(2):
                                c0 = half * 256
                                for kc in range(KC):
                                    nc.tensor.matmul(
                                        op[:, c0:c0 + 256],
                                        aw[:, kc, s * 128:(s + 1) * 128],
                                        wo_sb[:, kc, c0:c0 + 256],
                                        start=(kc == 0), stop=(kc == KC - 1))
                            os_ = sb.tile([128, 512], f32, tag="os", bufs=2,
                                          name="os_")
                            nc.any.tensor_copy(os_[:], op[:])
                            nc.sync.dma_start(
                                out_d.ap()[s0 + t0 + s * 128:
                                           s0 + t0 + (s + 1) * 128, :],
                                os_[:])

                # ---- schedule ----
                # tile_wait_until floors act as logical priorities: they stop
                # the tile scheduler from hoisting wo(b) matmuls (which wait
                # on AG(b) via their aw DMA) ahead of the next sequence's
                # attention in the in-order PE stream.
                if 1 in phases:
                    for i in range(4):        # tokens 0..1024 = seq 0
                        do_chunk(i, _rep == 0)
                if 2 in phases:
                    do_attn(0)                # AG(0) hides under phase 1
                if 1 in phases:
                    for i in range(4, NCH):
                        do_chunk(i, _rep == 0)
                if 3 in phases:
                    wo_sb = sb.tile([128, KC, QW], bf16, tag="bigw",
                                    name="wo_sb")
                    nc.sync.dma_start(wo_sb[:], wo_d.ap())
                    prefetch_aw(0)
                if 2 in phases:
                    with tc.tile_wait_until(1):
                        do_attn(1)
                    if 3 in phases:
                        with tc.tile_wait_until(2):
                            do_wo(0, wo_sb)
                            prefetch_aw(1)
                    with tc.tile_wait_until(3):
                        do_attn(2)
                    with tc.tile_wait_until(4):
                        do_attn(3)   # early: AG(3) hides under wo(1)+wo(2)
                    if 3 in phases:
                        with tc.tile_wait_until(5):
                            do_wo(1, wo_sb)
                            prefetch_aw(2)
                        with tc.tile_wait_until(6):
                            do_wo(2, wo_sb)
                            prefetch_aw(3)
                        with tc.tile_wait_until(7):
                            do_wo(3, wo_sb)

    nc.compile()
    return nc


def _host_prep(x, wq, wk, wv, wo, positions):
    """Per-core input maps: slice per head group, permute rope pairs, cast bf16."""
    # rope pair permutation within each head: evens then odds
    perm = np.concatenate([np.arange(0, HD, 2), np.arange(1, HD, 2)])

    inv_freq = 1.0 / (THETA ** (np.arange(64, dtype=np.float64) * 2.0 / HD))
    ang = positions.astype(np.float64)[None, :] * inv_freq[:, None]  # [64, T]
    cos_t = np.zeros((64, TP), np.float32)
    sin_t = np.zeros((64, TP), np.float32)
    cos_t[:, :T] = np.cos(ang)
    sin_t[:, :T] = np.sin(ang)

    tri = np.where(np.arange(128)[None, :] >= np.arange(128)[:, None],
                   1.0, 0.0).astype(BF16)

    # x -> chunk-major [128, NCH, KC, CHUNK]: one fat DMA per token chunk
    xT = np.zeros((DIM, TP), BF16)
    xT[:, :T] = x.T.astype(BF16)
    xc = np.ascontiguousarray(
        xT.reshape(KC, 128, NCH, CHUNK).transpose(1, 2, 0, 3))

    def shard_w(w_full, cols, permute):
        ws = w_full[:, cols].astype(np.float64)
        if permute is not None:
            nh = ws.shape[1] // HD
            ws = ws.reshape(DIM, nh, HD)[:, :, permute].reshape(DIM, nh * HD)
        return ws

    in_maps = []
    for c in range(NCORES):
        qcols = slice(c * QW, (c + 1) * QW)
        kcols = slice(c * HD, (c + 1) * HD)
        wq_c = shard_w(wq, qcols, perm) * SCALE
        wk_c = shard_w(wk, kcols, perm)
        wv_c = wv[:, kcols].astype(np.float64)
        wo_c = wo[:, qcols].astype(np.float64)

        def lay(wm):  # [DIM, n] -> [128, KC, n] with dim = a*128+p
            n = wm.shape[1]
            return np.ascontiguousarray(
                wm.reshape(KC, 128, n).transpose(1, 0, 2).astype(BF16))

        in_maps.append({
            "xT": xc,
            "wq": lay(wq_c),
            "wk": lay(wk_c),
            "wv": lay(wv_c),
            "wo": lay(wo_c),
            "cost": cos_t,
            "sint": sin_t,
            "tri": tri,
        })
    return in_maps


def _get_program():
    if "nc" not in _CACHE:
        _CACHE["nc"] = _build_program()
    return _CACHE["nc"]


def kernel(x, wq, wk, wv, wo, positions, _trace=False):
    from concourse import bass_utils

    nc = _get_program()
    in_maps = _host_prep(np.asarray(x), np.asarray(wq), np.asarray(wk),
                         np.asarray(wv), np.asarray(wo), np.asarray(positions))
    res = bass_utils.run_bass_kernel_spmd(
        nc, in_maps, core_ids=list(range(NCORES)), trace=_trace)
    _CACHE["last_result"] = res
    out = np.concatenate([res.results[c]["out"] for c in range(NCORES)], axis=1)
    return np.ascontiguousarray(out.astype(np.float32))
